# revision 10
# baseline (speedup 1.0000x reference)
"""Trainium2 Bass kernel: low-rank (LoRA-style) linear with 2:4 soft-threshold
pruned weights, fp16 matmul / fp32 accumulate.

  wA = soft_threshold24(weight_A) * scale_A          # [IN, R]
  wB = soft_threshold24(weight_B) * scale_B          # [OUT, R]
  x_proj = f16(x) @ f16(wA)            (f32 accum)   # [N, R]
  out    = f16(x_proj) @ f16(wB).T + bias            # [N, OUT]

Sharding: data-parallel over the token dim across 8 cores (2048 tokens/core),
small weights replicated. No collectives.

v3 notes (trace-driven):
 - x input: SWDGE cast-DMA f32(HBM)->f16(SBUF), 1MB half-tiles so transposes
   start as soon as the first half lands.  First two tiles issue before the
   identity/bias setup on the gpsimd queue.
 - weights: contiguous p-major load (128 fat 8KB descriptors instead of 4096
   x 256B strided ones, which starved behind the x stream for 40us in v2).
   Threshold math is layout-agnostic; wA is re-blocked on-chip by a small
   f16 SBUF->SBUF DMA; wB is consumed straight from p-major by PE transposes
   writing wbt columns through a strided ACT copy.
 - f16 PE transposes (single-pass), PSUM f16, ACT copies halved.
 - out: per 1MB half-tile on the sync queue; PSUM->SBUF copies alternate
   DVE/ACT to balance the two copy engines.
"""

import sys

import numpy as np

if "/opt/trn_rl_repo" not in sys.path:
    sys.path.insert(0, "/opt/trn_rl_repo")

B, S, IN_F, OUT_F, RANK = 4, 4096, 4096, 4096, 64
N_CORES = 8
N_TOK = B * S                   # 16384
T_CORE = N_TOK // N_CORES       # 2048 tokens per core
P = 128
TT = 2                          # token tiles per group
GTOK = TT * P                   # 256 tokens per group
N_GRP = T_CORE // GTOK          # 8 groups per core
N_IB = IN_F // P                # 32 input-feature blocks
MM2_N = 512
N_OB = OUT_F // MM2_N           # 8 output column groups

_CACHE = {}


def _soft_threshold_pmaj(nc, ve, pool, stage, scale, out_f16_pm):
    """out_f16_pm = f16(soft_threshold24(w) * scale), all in p-major layout
    [P, 32, RANK] where partition p holds rows 32p..32p+31 (elementwise, so
    layout only has to match between stage and out)."""
    import concourse.mybir as mybir

    f32 = mybir.dt.float32
    nb = stage.shape[1]
    amin = mybir.AluOpType.min
    amx = mybir.AluOpType.max

    wfh = stage[:]
    wneg = pool.tile([P, nb, RANK], f32, tag="wneg", name="wneg")
    ve.tensor_scalar_mul(wneg[:], wfh, -1.0)
    # |w| in one full-size op, then view quarters of the group-of-4 axis
    aw = pool.tile([P, nb, RANK], f32, tag="awabs", name="awabs")
    ve.tensor_tensor(aw[:], wfh, wneg[:], op=amx)
    a4 = aw[:].rearrange("p b (g q) -> p b g q", q=4)
    ab = [a4[:, :, :, j : j + 1] for j in range(4)]
    ash = [P, nb, RANK // 4, 1]
    m1 = pool.tile(ash, f32, tag="m1", name="m1")
    M1 = pool.tile(ash, f32, tag="M1", name="M1")
    m2 = pool.tile(ash, f32, tag="m2", name="m2")
    M2 = pool.tile(ash, f32, tag="M2", name="M2")
    ve.tensor_tensor(m1[:], ab[0], ab[1], op=amin)
    ve.tensor_tensor(M1[:], ab[0], ab[1], op=amx)
    ve.tensor_tensor(m2[:], ab[2], ab[3], op=amin)
    ve.tensor_tensor(M2[:], ab[2], ab[3], op=amx)
    # 2nd smallest of the 4 = min(max(m1, m2), min(M1, M2))
    t = pool.tile(ash, f32, tag="tq", name="t")
    ve.tensor_tensor(m1[:], m1[:], m2[:], op=amx)
    ve.tensor_tensor(M1[:], M1[:], M2[:], op=amin)
    ve.tensor_tensor(t[:], m1[:], M1[:], op=amin)
    # t4: threshold broadcast over the group-of-4 axis
    t4 = pool.tile([P, nb, RANK], f32, tag="t4", name="t4")
    h4 = t4[:].rearrange("p b (g q) -> p b g q", q=4)
    for j in range(4):
        ve.tensor_copy(h4[:, :, :, j : j + 1], t[:])
    # s = w - clip(w, -t, t);  -t4 reuses wneg's slot
    nt4 = pool.tile([P, nb, RANK], f32, tag="wneg", name="nt4")
    ve.tensor_scalar_mul(nt4[:], t4[:], -1.0)
    thr = pool.tile([P, nb, RANK], f32, tag="awabs", name="wthr")
    th = thr[:]
    ve.tensor_tensor(th, wfh, t4[:], op=amin)
    ve.tensor_tensor(th, th, nt4[:], op=amx)
    ve.tensor_sub(th, wfh, th)
    if scale != 1.0:
        ve.tensor_scalar_mul(th, th, float(scale))
    ck = nb // 2
    for c in range(2):
        ve.tensor_copy(out_f16_pm[:, c * ck : (c + 1) * ck, :],
                       thr[:, c * ck : (c + 1) * ck, :])


def _build(scale_a, scale_b):
    import concourse.mybir as mybir
    import concourse.tile as tile
    from concourse import bacc
    from concourse.bass import ts
    from concourse.masks import make_identity

    f32, f16 = mybir.dt.float32, mybir.dt.float16

    nc = bacc.Bacc("TRN2", target_bir_lowering=False, debug=False,
                   enable_asserts=False)
    x_d = nc.dram_tensor("x", [T_CORE, IN_F], f32, kind="ExternalInput")
    wa_d = nc.dram_tensor("weight_A", [IN_F, RANK], f32, kind="ExternalInput")
    wb_d = nc.dram_tensor("weight_B", [OUT_F, RANK], f32, kind="ExternalInput")
    b_d = nc.dram_tensor("bias", [1, OUT_F], f32, kind="ExternalInput")
    o_d = nc.dram_tensor("out", [T_CORE, OUT_F], f32, kind="ExternalOutput")

    with tile.TileContext(nc) as tc:
        with (
            tc.tile_pool(name="const", bufs=1) as constp,
            tc.tile_pool(name="wtmp", bufs=1) as wtmp,
            tc.tile_pool(name="xin", bufs=5) as xinp,
            tc.tile_pool(name="xtp", bufs=3) as xtp,
            tc.tile_pool(name="outp", bufs=4) as outp,
            tc.tile_pool(name="proj", bufs=6) as projp,
            tc.tile_pool(name="pst", bufs=3, space="PSUM") as pst,
            tc.tile_pool(name="ps1", bufs=2, space="PSUM") as ps1p,
            tc.tile_pool(name="ps2", bufs=3, space="PSUM") as ps2p,
        ):
            # --- weight stage loads: contiguous p-major, fat descriptors,
            # on the sync queue which is otherwise idle until outputs flow.
            wstg_a = wtmp.tile([P, N_IB, RANK], f32, tag="wstg_a", name="wsa")
            nc.sync.dma_start(wstg_a[:],
                              wa_d[:].rearrange("(p c) r -> p c r", p=P))
            wstg_b = wtmp.tile([P, N_IB, RANK], f32, tag="wstg_b", name="wsb")
            nc.sync.dma_start(wstg_b[:],
                              wb_d[:].rearrange("(p c) r -> p c r", p=P))

            # --- x input: first two tiles before ident/bias setup
            x16 = []

            def load_x(i):
                xt = xinp.tile([P, IN_F], f16, name="x16")
                for h in range(2):
                    nc.gpsimd.dma_start(
                        xt[:, h * (IN_F // 2) : (h + 1) * (IN_F // 2)],
                        x_d[ts(i, P), h * (IN_F // 2) : (h + 1) * (IN_F // 2)])
                x16.append(xt)

            load_x(0)
            load_x(1)

            ident16 = constp.tile([P, P], f16)
            make_identity(nc, ident16[:])

            # wbt: wB.T (+ bias row); bias row filled by cast-DMA (f32->f16)
            wbt = constp.tile([RANK + 1, OUT_F], f16)
            nc.gpsimd.dma_start(wbt[RANK : RANK + 1, :], b_d[:])

            for i in range(2, 2 * N_GRP):
                load_x(i)

            # --- weight preprocessing on DVE (p-major, elementwise) ---
            wa16pm = wtmp.tile([P, N_IB, RANK], f16, tag="wa16pm", name="wapm")
            _soft_threshold_pmaj(nc, nc.vector, wtmp, wstg_a, scale_a, wa16pm)
            wa16 = constp.tile([P, N_IB, RANK], f16)
            waT = wtmp.tile([RANK, IN_F], f16, tag="waT", name="waT")

            wb16pm = wtmp.tile([P, N_IB, RANK], f16, tag="wb16pm", name="wbpm")
            _soft_threshold_pmaj(nc, nc.vector, wtmp, wstg_b, scale_b, wb16pm)

            xTs = {}

            def emit_transposes(g):
                """PE f16 transposes of group g + ACT copies PSUM->SBUF."""
                xT = xtp.tile([P, N_IB, GTOK], f16, name="xT")
                for q in range(N_IB // 4):
                    for tt in range(TT):
                        pt = pst.tile([P, 4 * P], f16, tag="ptx", name="pt")
                        for bb in range(4):
                            b = 4 * q + bb
                            nc.tensor.transpose(
                                pt[:, ts(bb, P)],
                                x16[g * TT + tt][:, ts(b, P)], ident16[:])
                        dst = xT[:, 4 * q : 4 * q + 4, ts(tt, P)]
                        nc.scalar.copy(dst,
                                       pt[:].rearrange("p (a b) -> p a b", a=4))
                xTs[g] = xT

            xpas = {}

            def emit_mm1(g):
                ps1 = ps1p.tile([RANK, GTOK], f32)
                for b in range(N_IB):
                    nc.tensor.matmul(ps1[:], wa16[:, b, :], xTs[g][:, b, :],
                                     start=(b == 0), stop=(b == N_IB - 1))
                xpa = projp.tile([RANK + 1, GTOK], f16)
                nc.vector.tensor_copy(out=xpa[0:RANK, :], in_=ps1[:])
                nc.vector.memset(xpa[RANK : RANK + 1, :], 1.0)
                xpas[g] = xpa

            # wbt columns i = 32p + c from p-major wb16pm via PE transpose;
            # ACT copies land 4 c-columns per psum tile through a strided AP.
            wbt_cols = wbt[0:RANK, :].rearrange("o (p c) -> o c p", c=N_IB)

            def emit_wbt():
                for q in range(N_IB // 4):
                    pw = pst.tile([P, 4 * P], f16, tag="ptx", name="pw")
                    for cc in range(4):
                        c = 4 * q + cc
                        nc.tensor.transpose(pw[:RANK, ts(cc, P)],
                                            wb16pm[:, c, :], ident16[:])
                    nc.scalar.copy(
                        wbt_cols[:, 4 * q : 4 * q + 4, :],
                        pw[:RANK, :].rearrange("o (c p) -> o c p", c=4))

            # wA re-block p-major -> block-major via PE double transpose:
            # pass 1 builds waT[r, i] (columns i = 32p + c, strided copies),
            # pass 2 transposes 128-column slabs of waT back to wa16 blocks.
            waT_cols = waT[:].rearrange("o (p c) -> o c p", c=N_IB)

            def emit_wat():
                for q in range(N_IB // 4):
                    pw = pst.tile([P, 4 * P], f16, tag="ptx", name="pw")
                    for cc in range(4):
                        c = 4 * q + cc
                        nc.tensor.transpose(pw[:RANK, ts(cc, P)],
                                            wa16pm[:, c, :], ident16[:])
                    nc.scalar.copy(
                        waT_cols[:, 4 * q : 4 * q + 4, :],
                        pw[:RANK, :].rearrange("o (c p) -> o c p", c=4))
                for q in range(N_IB // 4):
                    pw = pst.tile([P, 4 * P], f16, tag="ptx", name="pw")
                    for bb in range(4):
                        b = 4 * q + bb
                        nc.tensor.transpose(pw[:, ts(bb, RANK)],
                                            waT[:, ts(b, P)],
                                            ident16[:RANK, :RANK])
                    nc.scalar.copy(
                        wa16[:, 4 * q : 4 * q + 4, :],
                        pw[:, : 4 * RANK].rearrange("p (a r) -> p a r", a=4))

            def emit_mm2(g):
                for tt in range(TT):
                    i = g * TT + tt
                    for h in range(2):
                        ob = outp.tile([P, OUT_F // 2], f32, name="ob",
                                       tag="ob")
                        for jj in range(N_OB // 2):
                            j = h * (N_OB // 2) + jj
                            ps2 = ps2p.tile([P, MM2_N], f32, tag="ps2",
                                            name="ps2")
                            nc.tensor.matmul(ps2[:], xpas[g][:, ts(tt, P)],
                                             wbt[:, ts(j, MM2_N)],
                                             start=True, stop=True)
                            if j % 2 == 0:
                                nc.vector.tensor_copy(
                                    out=ob[:, ts(jj, MM2_N)], in_=ps2[:])
                            else:
                                nc.scalar.copy(ob[:, ts(jj, MM2_N)], ps2[:])
                        nc.sync.dma_start(
                            o_d[ts(i, P),
                                h * (OUT_F // 2) : (h + 1) * (OUT_F // 2)],
                            ob[:])

            # --- static schedule: transposes run ahead while weights
            # preprocess; mm1 follows wa16, mm2 follows wbt.
            emit_transposes(0)
            emit_transposes(1)
            emit_transposes(2)
            emit_wat()
            emit_mm1(0)
            emit_transposes(3)
            emit_wbt()
            emit_mm1(1)
            emit_mm2(0)
            emit_transposes(4)
            emit_mm1(2)
            emit_mm2(1)
            emit_transposes(5)
            emit_mm1(3)
            emit_mm2(2)
            emit_transposes(6)
            emit_mm1(4)
            emit_mm2(3)
            emit_transposes(7)
            emit_mm1(5)
            emit_mm2(4)
            emit_mm1(6)
            emit_mm2(5)
            emit_mm1(7)
            emit_mm2(6)
            emit_mm2(7)

    nc.compile()
    return nc


def get_nc(scale_a, scale_b):
    key = (float(scale_a), float(scale_b))
    if key not in _CACHE:
        _CACHE[key] = _build(*key)
    return _CACHE[key]


def kernel(x, weight_A, weight_B, bias, scale_A, scale_B):
    from concourse.bass_utils import run_bass_kernel_spmd

    x = np.ascontiguousarray(np.asarray(x, dtype=np.float32))
    wa = np.ascontiguousarray(np.asarray(weight_A, dtype=np.float32))
    wb = np.ascontiguousarray(np.asarray(weight_B, dtype=np.float32))
    bi = np.ascontiguousarray(np.asarray(bias, dtype=np.float32)).reshape(1, OUT_F)
    sa = float(np.asarray(scale_A))
    sb = float(np.asarray(scale_B))

    nc = get_nc(sa, sb)

    xf = x.reshape(N_TOK, IN_F)
    in_maps = [
        {
            "x": xf[c * T_CORE : (c + 1) * T_CORE],
            "weight_A": wa,
            "weight_B": wb,
            "bias": bi,
        }
        for c in range(N_CORES)
    ]
    res = run_bass_kernel_spmd(nc, in_maps, core_ids=list(range(N_CORES)))
    out = np.concatenate([r["out"] for r in res.results], axis=0)
    return out.reshape(B, S, OUT_F)
